# revision 4
# baseline (speedup 1.0000x reference)
"""Row softmax on 8 TRN2 NeuronCores (Bass/Tile, SPMD data-parallel).

The reference computes softmax(x - rowmean(x)) per row, with exp approximated
by a 100-term Taylor series of 2**frac plus exp2 of the integer part.  That is
mathematically softmax(x, axis=1) (softmax is shift invariant; the Taylor
series converges to f32 precision), so the kernel computes a plain row softmax
with the ScalarEngine's Exp activation.

Sharding: pure data parallel — 262144 rows split into 8 shards of 32768 rows,
one per core; each row's reduction is local to its core.

Device I/O is fp16 (host casts f32->fp16 before upload, widens after
download; rel err ~5e-4 vs 2e-2 tolerance): 16 MiB in + 16 MiB out per core.

Per-core layout: the 32768-row shard is viewed as [128 partitions, 256 rows,
256 cols]; partition p owns 256 consecutive rows.  Inputs stream on the sync
HWDGE queue, outputs on the gpsimd SWDGE queue.

Compute per tile of R rows/partition: one big ACT Exp over all R rows (fp16,
1 elem/cycle/lane); a DVE add-tree 256->128->64->32 (fp16 TENSOR_TENSOR runs
2x) + TENSOR_REDUCE for the row sums; one DVE reciprocal (f32) and a tiny
broadcast copy to [R,2] fp16; then the normalize: rows are split between a
single full-tile DVE TENSOR_TENSOR mul (in1 = the [R,2] recip broadcast with
stride-0 middle dims — only the innermost step matters for the 2x packed
mode) and, for `k` column-slices j per tile, a ScalarE Copy-activation with
per-partition scale = rbf[:, j] (each partition holds a *different* row at
column-slice j, so one ACT instruction normalizes 128 rows).  The k per tile
balances ACT (exp + k*0.4us) against DVE (tree + mul) at ~72-76us each.

The h1/h2/h3 tree scratch lives inside the dead xt input tile (xt has no
reader after the single big exp), saving SBUF and allowing 5-deep xt/et
pools.  Small leading tiles start compute sooner; small trailing tiles
shorten the end-of-pipeline drain.
"""

import numpy as np

N, C = 262144, 256
N_CORES = 8
ROWS_PER_CORE = N // N_CORES        # 32768
P = 128                             # SBUF partitions
ROWS_PER_PART = ROWS_PER_CORE // P  # 256 rows owned by each partition

_NC_CACHE = {}


def _get_nc():
    if "nc" in _NC_CACHE:
        return _NC_CACHE["nc"]
    from contextlib import ExitStack

    import concourse.tile as tile
    from concourse import bacc, mybir

    nc = bacc.Bacc(
        "TRN2", target_bir_lowering=False, debug=False,
        enable_asserts=False, num_devices=N_CORES,
    )
    x_h = nc.dram_tensor("x", [ROWS_PER_CORE, C], mybir.dt.float16,
                         kind="ExternalInput")
    o_h = nc.dram_tensor("out", [ROWS_PER_CORE, C], mybir.dt.float16,
                         kind="ExternalOutput")
    x_prc = x_h.ap().rearrange("(p r) c -> p r c", p=P)
    o_prc = o_h.ap().rearrange("(p r) c -> p r c", p=P)

    # Tile schedule: (rows, act_mul_rows).  Small leading tiles start compute
    # sooner (pipeline fill); small trailing tiles shorten the drain (the last
    # tile's exp->tree->recip->mul->DMA chain).  act_mul_rows = column-slices
    # whose normalize runs on ScalarE (Copy with per-partition scale), which
    # offloads the DVE just enough to balance the two engines.
    SEGS = [(4, 0), (8, 0), (16, 2),
            (32, 4), (32, 4), (32, 4), (32, 4), (32, 4), (32, 4),
            (24, 2), (8, 0), (4, 0)]
    assert sum(r for r, _ in SEGS) == ROWS_PER_PART

    W = 2                  # recip replication width for the DVE mul's in1

    with tile.TileContext(nc) as tc, ExitStack() as ctx:
        xp = ctx.enter_context(tc.tile_pool(name="xp", bufs=5))
        ep = ctx.enter_context(tc.tile_pool(name="ep", bufs=5))
        sp = ctx.enter_context(tc.tile_pool(name="sp", bufs=4))
        cp = ctx.enter_context(tc.tile_pool(name="cp", bufs=1))
        # memset zero-bias tile: a float bias would be materialized as a
        # const tensor whose DMA load serializes ahead of the first input
        # DMA on the HWDGE queue (~6us of kernel-start latency).
        bias0 = cp.tile([P, 1], mybir.dt.float32)
        nc.vector.memset(bias0[:], 0.0)
        # Dummy tiny exp: forces the Exp ACT_TABLE_LOAD to happen now,
        # overlapped with the first input DMA, instead of serialized
        # between the first DMA-wait and the first real exp (~2.3us).
        warm = cp.tile([P, 1], mybir.dt.float16)
        nc.scalar.activation(warm[:], bias0[:],
                             mybir.ActivationFunctionType.Exp, bias=bias0[:])
        row0 = 0
        for R_t, k in SEGS:
            B = R_t - k            # rows normalized by the DVE mul
            xt = xp.tile([P, R_t * C], mybir.dt.float16, tag="xt", bufs=5)
            # inputs issue from sync (HWDGE); outputs from gpsimd (SWDGE)
            # so input DMAs never queue behind output DMAs waiting on the
            # DVE chain (HWDGE is FIFO per issuing engine).
            nc.sync.dma_start(
                xt[:].rearrange("p (r c) -> p r c", c=C),
                x_prc[:, row0:row0 + R_t, :],
            )
            et = ep.tile([P, R_t * C], mybir.dt.float16, tag="et", bufs=5)
            st = sp.tile([P, R_t], mybir.dt.float32, tag="st", bufs=4)
            e3 = et[:].rearrange("p (r c) -> p r c", c=C)
            nc.scalar.activation(
                et[:], xt[:],
                mybir.ActivationFunctionType.Exp, bias=bias0[:],
            )
            # add-tree scratch inside the now-dead xt tile (fp16):
            # h1 [R,128] at cols 0..128R, then h2 [R,64], h3 [R,32], h4 [R,16]
            x3 = xt[:].rearrange("p (r c) -> p r c", c=C)
            h1 = x3[:, :, 0:128]
            h2 = x3[:, :, 128:192]
            h3 = x3[:, :, 192:224]
            h4 = x3[:, :, 224:240]
            nc.vector.tensor_add(h1, e3[:, :, 0:128], e3[:, :, 128:256])
            nc.vector.tensor_add(h2, h1[:, :, 0:64], h1[:, :, 64:128])
            # h3+h4 run on the otherwise-idle GpSimd Q7 cores (~2.2ns/elem
            # 2-input) to take ~19us of tree work off the bottleneck DVE;
            # the DVE picks the chain back up at the cheap 16-wide reduce.
            nc.gpsimd.tensor_add(h3, h2[:, :, 0:32], h2[:, :, 32:64])
            nc.gpsimd.tensor_add(h4, h3[:, :, 0:16], h3[:, :, 16:32])
            nc.vector.reduce_sum(st[:], h4, axis=mybir.AxisListType.X)
            # 1/sum: f32 straight (feeds the ACT Copy-scale rows), plus a
            # tiny fp16 copy replicated to W consecutive elements so the
            # DVE normalization runs as ONE TENSOR_TENSOR in the 2x packed
            # mode (in1 innermost step 1; stride-0 outer dims are fine).
            rbf = sp.tile([P, R_t], mybir.dt.float32, tag="rbf", bufs=4)
            with nc.allow_low_precision(reason="fp16 softmax kernel"):
                nc.vector.reciprocal(rbf[:], st[:])
            rb = sp.tile([P, R_t, W], mybir.dt.float16, tag="rb", bufs=4)
            nc.vector.tensor_copy(
                rb[:, 0:B, :],
                rbf[:, 0:B, None].broadcast_to((P, B, W)),
            )
            e4 = et[:, 0:B * C].rearrange("p (r k w) -> p r k w", w=W,
                                          k=C // W)
            in1 = rb[:, 0:B, None, :].broadcast_to((P, B, C // W, W))
            nc.vector.tensor_mul(e4, e4, in1)
            for j in range(B, R_t):
                # partition p's column-slice j is row p*256+j': one ACT
                # Copy normalizes 128 rows with per-partition scale 1/sum
                nc.scalar.mul(e3[:, j, :], e3[:, j, :], rbf[:, j:j + 1])
            nc.gpsimd.dma_start(o_prc[:, row0:row0 + R_t, :], e3)
            row0 += R_t
    nc.compile()
    _NC_CACHE["nc"] = nc
    return nc


def _install_ntff_hook():
    """Make the optional antenv.axon_hooks module available so the
    trace=True / BASS_TRACE path of run_bass_kernel_spmd works under axon
    (the image's antenv package lacks axon_hooks; boot() skips the NTFF
    hook registration silently in that case)."""
    import sys
    import types

    try:
        import antenv.axon_hooks  # noqa: F401
    except ImportError:
        try:
            import antenv
        except ImportError:
            return
        mod = types.ModuleType("antenv.axon_hooks")
        holder = {}
        mod.set_axon_ntff_profile_hook = lambda h: holder.__setitem__("h", h)
        mod.get_axon_ntff_profile_hook = lambda: holder.get("h")
        sys.modules["antenv.axon_hooks"] = mod
        antenv.axon_hooks = mod
    from antenv.axon_hooks import (
        get_axon_ntff_profile_hook,
        set_axon_ntff_profile_hook,
    )

    if get_axon_ntff_profile_hook() is None:
        try:
            from trn_agent_boot.trn_boot import _ntff_profile_via_ctypes

            set_axon_ntff_profile_hook(
                _ntff_profile_via_ctypes("/opt/axon/libaxon_pjrt.so")
            )
        except Exception:
            pass


def _build_per_device_runner(nc):
    """Per-device dispatch in HBM-domain-interleaved order.

    Host->device uploads serialize, so with a single global dispatch each
    even core's NEFF executes exactly while its HBM-domain partner's
    (core+1) input upload streams into the same HBM stack.  Dispatching
    per-device executions in order 0,2,4,6,1,3,5,7 makes the upload that
    overlaps core i's execution always target a different HBM domain.
    """
    import jax
    import jax.numpy as jnp
    from concourse import bass2jax, mybir

    bass2jax.install_neuronx_cc_hook()

    partition_name = (
        nc.partition_id_tensor.name if nc.partition_id_tensor else None
    )
    in_names, out_names, out_avals = [], [], []
    for alloc in nc.m.functions[0].allocations:
        if not isinstance(alloc, mybir.MemoryLocationSet):
            continue
        assert alloc.memorylocations
        name = alloc.memorylocations[0].name
        if alloc.kind == "ExternalInput":
            if name != partition_name:
                in_names.append(name)
        elif alloc.kind == "ExternalOutput":
            assert alloc.tensor_shape is not None and alloc.dtype is not None
            out_names.append(name)
            out_avals.append(
                jax.core.ShapedArray(
                    tuple(alloc.tensor_shape), mybir.dt.np(alloc.dtype)
                )
            )
    n_params = len(in_names)
    all_in_names = tuple(in_names) + tuple(out_names)
    if partition_name is not None:
        # supplied as the last operand via PartitionIdOp, mirroring
        # run_bass_via_pjrt; this program never reads it (no collectives).
        all_in_names = all_in_names + (partition_name,)

    def _body(*args):
        operands = list(args)
        if partition_name is not None:
            operands.append(bass2jax.partition_id_tensor())
        outs = bass2jax._bass_exec_p.bind(
            *operands,
            out_avals=tuple(out_avals),
            in_names=all_in_names,
            out_names=tuple(out_names),
            lowering_input_output_aliases=(),
            sim_require_finite=True,
            sim_require_nnan=True,
            nc=nc,
        )
        return tuple(outs)

    donate = tuple(range(n_params, n_params + len(out_names)))
    jitted = jax.jit(_body, donate_argnums=donate, keep_unused=True)

    devs = jax.devices()[:N_CORES]
    zeros_makers = {
        d: jax.jit(
            lambda: tuple(jnp.zeros(a.shape, a.dtype) for a in out_avals),
            out_shardings=jax.sharding.SingleDeviceSharding(devs[d]),
        )
        for d in range(N_CORES)
    }

    def run(in_maps, order=(0, 2, 4, 6, 1, 3, 5, 7)):
        futures = {}
        for d in order:
            args = [
                jax.device_put(np.asarray(in_maps[d][n]), devs[d])
                for n in in_names
            ]
            zeros = zeros_makers[d]()  # created on-device: no H2D traffic
            futures[d] = jitted(*args, *zeros)
        return [
            {n: np.asarray(futures[d][i]) for i, n in enumerate(out_names)}
            for d in range(len(in_maps))
        ]

    return run


def _run(x, **spmd_kwargs):
    _install_ntff_hook()
    nc = _get_nc()
    x = np.asarray(x)
    assert x.shape == (N, C), x.shape
    x16 = np.ascontiguousarray(x, dtype=np.float16)
    shards = np.split(x16, N_CORES, axis=0)
    in_maps = [{"x": np.ascontiguousarray(s)} for s in shards]

    if not spmd_kwargs:
        try:
            if "runner" not in _NC_CACHE:
                _NC_CACHE["runner"] = _build_per_device_runner(nc)
            results = _NC_CACHE["runner"](in_maps)
            out = np.concatenate(
                [r["out"] for r in results], axis=0
            ).astype(np.float32)
            return out, None
        except Exception:
            pass  # fall back to the stock global-dispatch path

    from concourse.bass_utils import run_bass_kernel_spmd

    res = run_bass_kernel_spmd(
        nc, in_maps, core_ids=list(range(N_CORES)), **spmd_kwargs
    )
    out = np.concatenate(
        [np.asarray(res.results[i]["out"]) for i in range(N_CORES)], axis=0
    ).astype(np.float32)
    return out, res


def kernel(x):
    return _run(x)[0]


# revision 12
# speedup vs baseline: 1.1133x; 1.1133x over previous
"""Row softmax on 8 TRN2 NeuronCores (Bass/Tile, SPMD data-parallel).

The reference computes softmax(x - rowmean(x)) per row, with exp approximated
by a 100-term Taylor series of 2**frac plus exp2 of the integer part.  That is
mathematically softmax(x, axis=1) (softmax is shift invariant; the Taylor
series converges to f32 precision), so the kernel computes a plain row softmax
with the ScalarEngine's Exp activation.

Sharding: pure data parallel — 262144 rows split into 8 shards of 32768 rows,
one per core; each row's reduction is local to its core.

Device I/O is fp16 (host casts f32->fp16 before upload, widens after
download; rel err ~5e-4 vs 2e-2 tolerance): 16 MiB in + 16 MiB out per core.

Per-core layout: the 32768-row shard is viewed as [128 partitions, 256 rows,
256 cols]; partition p owns 256 consecutive rows.  Inputs stream on the sync
HWDGE queue, outputs on the gpsimd SWDGE queue.

Compute per tile of R rows/partition: one big ACT Exp over all R rows (fp16,
1 elem/cycle/lane); a DVE add-tree 256->128->64->32 (fp16 TENSOR_TENSOR runs
2x) + TENSOR_REDUCE for the row sums; one DVE reciprocal (f32) and a tiny
broadcast copy to [R,2] fp16; then the normalize: rows are split between a
single full-tile DVE TENSOR_TENSOR mul (in1 = the [R,2] recip broadcast with
stride-0 middle dims — only the innermost step matters for the 2x packed
mode) and, for `k` column-slices j per tile, a ScalarE Copy-activation with
per-partition scale = rbf[:, j] (each partition holds a *different* row at
column-slice j, so one ACT instruction normalizes 128 rows).  The k per tile
balances ACT (exp + k*0.4us) against DVE (tree + mul) at ~72-76us each.

The h1/h2/h3 tree scratch lives inside the dead xt input tile (xt has no
reader after the single big exp), saving SBUF and allowing 5-deep xt/et
pools.  Small leading tiles start compute sooner; small trailing tiles
shorten the end-of-pipeline drain.
"""

import numpy as np

N, C = 262144, 256
N_CORES = 8
ROWS_PER_CORE = N // N_CORES        # 32768
P = 128                             # SBUF partitions
ROWS_PER_PART = ROWS_PER_CORE // P  # 256 rows owned by each partition

_NC_CACHE = {}


def _get_nc():
    if "nc" in _NC_CACHE:
        return _NC_CACHE["nc"]
    from contextlib import ExitStack

    import concourse.tile as tile
    from concourse import bacc, mybir

    nc = bacc.Bacc(
        "TRN2", target_bir_lowering=False, debug=False,
        enable_asserts=False, num_devices=N_CORES,
    )
    x_h = nc.dram_tensor("x", [ROWS_PER_CORE, C], mybir.dt.float16,
                         kind="ExternalInput")
    o_h = nc.dram_tensor("out", [ROWS_PER_CORE, C], mybir.dt.float16,
                         kind="ExternalOutput")
    x_prc = x_h.ap().rearrange("(p r) c -> p r c", p=P)
    o_prc = o_h.ap().rearrange("(p r) c -> p r c", p=P)

    # Tile schedule: (rows, act_mul_rows).  Small leading tiles start compute
    # sooner (pipeline fill); small trailing tiles shorten the drain (the last
    # tile's exp->tree->recip->mul->DMA chain).  act_mul_rows = column-slices
    # whose normalize runs on ScalarE (Copy with per-partition scale), which
    # offloads the DVE just enough to balance the two engines.
    SEGS = [(4, 0), (8, 0), (16, 2),
            (32, 3), (32, 3), (32, 3), (32, 3), (32, 3), (32, 3),
            (24, 0), (8, 0), (4, 0)]
    assert sum(r for r, _ in SEGS) == ROWS_PER_PART

    W = 2                  # recip replication width for the DVE mul's in1

    with tile.TileContext(nc) as tc, ExitStack() as ctx:
        # two pools only: each pool close emits multi-engine barrier rounds
        # in the (serialized, on-the-clock) teardown
        xp = ctx.enter_context(tc.tile_pool(name="xp", bufs=5))
        sp = ctx.enter_context(tc.tile_pool(name="sp", bufs=4))
        # memset zero-bias tile: a float bias would be materialized as a
        # const tensor whose DMA load serializes ahead of the first input
        # DMA on the HWDGE queue (~6us of kernel-start latency).
        bias0 = sp.tile([P, 1], mybir.dt.float32, tag="bias0", bufs=1)
        nc.vector.memset(bias0[:], 0.0)
        # Dummy tiny exp: forces the Exp ACT_TABLE_LOAD to happen now,
        # overlapped with the first input DMA, instead of serialized
        # between the first DMA-wait and the first real exp (~2.3us).
        warm = sp.tile([P, 1], mybir.dt.float16, tag="warm", bufs=1)
        nc.scalar.activation(warm[:], bias0[:],
                             mybir.ActivationFunctionType.Exp, bias=bias0[:])
        row0 = 0
        for R_t, k in SEGS:
            B = R_t - k            # rows normalized by the DVE mul
            xt = xp.tile([P, R_t * C], mybir.dt.float16, tag="xt", bufs=5)
            # inputs issue from sync (HWDGE); outputs from gpsimd (SWDGE)
            # so input DMAs never queue behind output DMAs waiting on the
            # DVE chain (HWDGE is FIFO per issuing engine).
            nc.sync.dma_start(
                xt[:].rearrange("p (r c) -> p r c", c=C),
                x_prc[:, row0:row0 + R_t, :],
            )
            et = xp.tile([P, R_t * C], mybir.dt.float16, tag="et", bufs=5)
            st = sp.tile([P, R_t], mybir.dt.float32, tag="st", bufs=4)
            e3 = et[:].rearrange("p (r c) -> p r c", c=C)
            nc.scalar.activation(
                et[:], xt[:],
                mybir.ActivationFunctionType.Exp, bias=bias0[:],
            )
            # add-tree scratch inside the now-dead xt tile (fp16):
            # h1 [R,128] at cols 0..128R, then h2 [R,64], h3 [R,32], h4 [R,16]
            x3 = xt[:].rearrange("p (r c) -> p r c", c=C)
            h1 = x3[:, :, 0:128]
            h2 = x3[:, :, 128:192]
            h3 = x3[:, :, 192:224]
            h4 = x3[:, :, 224:240]
            nc.vector.tensor_add(h1, e3[:, :, 0:128], e3[:, :, 128:256])
            nc.vector.tensor_add(h2, h1[:, :, 0:64], h1[:, :, 64:128])
            nc.vector.tensor_add(h3, h2[:, :, 0:32], h2[:, :, 32:64])
            nc.vector.tensor_add(h4, h3[:, :, 0:16], h3[:, :, 16:32])
            nc.vector.reduce_sum(st[:], h4, axis=mybir.AxisListType.X)
            # 1/sum as fp16, replicated to W consecutive elements so the
            # DVE normalization runs as ONE TENSOR_TENSOR in the 2x packed
            # mode (in1 innermost step 1; stride-0 outer dims are fine).
            # The ACT Copy-scale rows read the same fp16 rb[:, j, 0:1].
            rb = sp.tile([P, R_t, W], mybir.dt.float16, tag="rb", bufs=4)
            with nc.allow_low_precision(reason="fp16 softmax kernel"):
                nc.vector.reciprocal(rb[:, 0:B, 0:1], st[:, 0:B][:, :, None])
            nc.vector.tensor_copy(
                rb[:, 0:B, 1:W],
                rb[:, 0:B, 0:1].broadcast_to((P, B, W - 1)),
            )
            if k:
                # the ACT Copy-scale rows need an f32 scale operand (the
                # BIR verifier rejects fp16 scale APs on Activation)
                rbs = sp.tile([P, R_t], mybir.dt.float32, tag="rbs", bufs=4)
                nc.vector.reciprocal(rbs[:, B:R_t], st[:, B:R_t])
            e4 = et[:, 0:B * C].rearrange("p (r k w) -> p r k w", w=W,
                                          k=C // W)
            in1 = rb[:, 0:B, None, :].broadcast_to((P, B, C // W, W))
            nc.vector.tensor_mul(e4, e4, in1)
            for j in range(B, R_t):
                # partition p's column-slice j is row p*256+j: one ACT
                # Copy normalizes 128 rows with per-partition scale 1/sum
                nc.scalar.mul(e3[:, j, :], e3[:, j, :], rbs[:, j:j + 1])
            nc.gpsimd.dma_start(o_prc[:, row0:row0 + R_t, :], e3)
            row0 += R_t
    nc.compile()
    _NC_CACHE["nc"] = nc
    return nc


def _install_ntff_hook():
    """Make the optional antenv.axon_hooks module available so the
    trace=True / BASS_TRACE path of run_bass_kernel_spmd works under axon
    (the image's antenv package lacks axon_hooks; boot() skips the NTFF
    hook registration silently in that case)."""
    import sys
    import types

    try:
        import antenv.axon_hooks  # noqa: F401
    except ImportError:
        try:
            import antenv
        except ImportError:
            return
        mod = types.ModuleType("antenv.axon_hooks")
        holder = {}
        mod.set_axon_ntff_profile_hook = lambda h: holder.__setitem__("h", h)
        mod.get_axon_ntff_profile_hook = lambda: holder.get("h")
        sys.modules["antenv.axon_hooks"] = mod
        antenv.axon_hooks = mod
    from antenv.axon_hooks import (
        get_axon_ntff_profile_hook,
        set_axon_ntff_profile_hook,
    )

    if get_axon_ntff_profile_hook() is None:
        try:
            from trn_agent_boot.trn_boot import _ntff_profile_via_ctypes

            set_axon_ntff_profile_hook(
                _ntff_profile_via_ctypes("/opt/axon/libaxon_pjrt.so")
            )
        except Exception:
            pass


def _build_per_device_runner(nc):
    """Per-device dispatch in HBM-domain-interleaved order.

    Host->device uploads serialize, so with a single global dispatch each
    even core's NEFF executes exactly while its HBM-domain partner's
    (core+1) input upload streams into the same HBM stack.  Dispatching
    per-device executions in order 0,2,4,6,1,3,5,7 makes the upload that
    overlaps core i's execution always target a different HBM domain.
    """
    import jax
    import jax.numpy as jnp
    from concourse import bass2jax, mybir

    bass2jax.install_neuronx_cc_hook()

    partition_name = (
        nc.partition_id_tensor.name if nc.partition_id_tensor else None
    )
    in_names, out_names, out_avals = [], [], []
    for alloc in nc.m.functions[0].allocations:
        if not isinstance(alloc, mybir.MemoryLocationSet):
            continue
        assert alloc.memorylocations
        name = alloc.memorylocations[0].name
        if alloc.kind == "ExternalInput":
            if name != partition_name:
                in_names.append(name)
        elif alloc.kind == "ExternalOutput":
            assert alloc.tensor_shape is not None and alloc.dtype is not None
            out_names.append(name)
            out_avals.append(
                jax.core.ShapedArray(
                    tuple(alloc.tensor_shape), mybir.dt.np(alloc.dtype)
                )
            )
    n_params = len(in_names)
    all_in_names = tuple(in_names) + tuple(out_names)
    if partition_name is not None:
        # supplied as the last operand via PartitionIdOp, mirroring
        # run_bass_via_pjrt; this program never reads it (no collectives).
        all_in_names = all_in_names + (partition_name,)

    def _body(*args):
        operands = list(args)
        if partition_name is not None:
            operands.append(bass2jax.partition_id_tensor())
        outs = bass2jax._bass_exec_p.bind(
            *operands,
            out_avals=tuple(out_avals),
            in_names=all_in_names,
            out_names=tuple(out_names),
            lowering_input_output_aliases=(),
            sim_require_finite=True,
            sim_require_nnan=True,
            nc=nc,
        )
        return tuple(outs)

    donate = tuple(range(n_params, n_params + len(out_names)))
    jitted = jax.jit(_body, donate_argnums=donate, keep_unused=True)

    devs = jax.devices()[:N_CORES]
    zeros_makers = {
        d: jax.jit(
            lambda: tuple(jnp.zeros(a.shape, a.dtype) for a in out_avals),
            out_shardings=jax.sharding.SingleDeviceSharding(devs[d]),
        )
        for d in range(N_CORES)
    }

    def run(in_maps, order=(0, 2, 4, 6, 1, 3, 5, 7)):
        futures = {}
        for d in order:
            args = [
                jax.device_put(np.asarray(in_maps[d][n]), devs[d])
                for n in in_names
            ]
            zeros = zeros_makers[d]()  # created on-device: no H2D traffic
            futures[d] = jitted(*args, *zeros)
        return [
            {n: np.asarray(futures[d][i]) for i, n in enumerate(out_names)}
            for d in range(len(in_maps))
        ]

    return run


def _run(x, **spmd_kwargs):
    _install_ntff_hook()
    nc = _get_nc()
    x = np.asarray(x)
    assert x.shape == (N, C), x.shape
    x16 = np.ascontiguousarray(x, dtype=np.float16)
    shards = np.split(x16, N_CORES, axis=0)
    in_maps = [{"x": np.ascontiguousarray(s)} for s in shards]

    if not spmd_kwargs:
        try:
            if "runner" not in _NC_CACHE:
                _NC_CACHE["runner"] = _build_per_device_runner(nc)
            results = _NC_CACHE["runner"](in_maps)
            out = np.concatenate(
                [r["out"] for r in results], axis=0
            ).astype(np.float32)
            return out, None
        except Exception:
            pass  # fall back to the stock global-dispatch path

    from concourse.bass_utils import run_bass_kernel_spmd

    res = run_bass_kernel_spmd(
        nc, in_maps, core_ids=list(range(N_CORES)), **spmd_kwargs
    )
    out = np.concatenate(
        [np.asarray(res.results[i]["out"]) for i in range(N_CORES)], axis=0
    ).astype(np.float32)
    return out, res


def kernel(x):
    return _run(x)[0]


# revision 13
# speedup vs baseline: 1.1207x; 1.0067x over previous
"""Row softmax on 8 TRN2 NeuronCores (Bass/Tile, SPMD data-parallel).

The reference computes softmax(x - rowmean(x)) per row, with exp approximated
by a 100-term Taylor series of 2**frac plus exp2 of the integer part.  That is
mathematically softmax(x, axis=1) (softmax is shift invariant; the Taylor
series converges to f32 precision), so the kernel computes a plain row softmax
with the ScalarEngine's Exp activation.

Sharding: pure data parallel — 262144 rows split into 8 shards of 32768 rows,
one per core; each row's reduction is local to its core.

The kernel is bound jointly by device DMA (fp16 I/O: 16 MiB in + 16 MiB out
per core over a ~435 GB/s SBUF fabric) and by the two elementwise engines:
ACT does the exp (~55us floor), DVE does the row-sum tree + normalize mul
(~80us including per-instruction overheads).  Host casts x f32->fp16 before
upload and widens the fp16 result after download (rel err ~5e-4, tol 2e-2).

Per-core layout: the 32768-row shard is viewed as [128 partitions, 256 rows,
256 cols]; partition p owns 256 consecutive rows, so every DMA moves large
per-partition-contiguous chunks.  Inputs stream on the sync HWDGE queue,
outputs on the gpsimd SWDGE queue.

Compute per tile: one big ACT Exp (fp16 in/out, 1 elem/cycle/lane), then the
row sums: `act_rows` per tile come from per-row ACT exp+accum (the
ACTIVATION_READ_ACCUMULATOR path, ~0.5us/row marginal, balancing the two
engine queues), the rest from a DVE add-tree 256->128->64->32->16 (fp16
TENSOR_TENSOR runs the 2x packed mode; InstTensorReduce has no fast mode so
the tree ends in a cheap 16-wide reduce).  Then one DVE reciprocal -> fp16,
replicated to W=2 consecutive elements so the normalization runs as ONE
full-tile TENSOR_TENSOR in 2x packed mode (in1 = [P,(R),(C/W,0),(W,1)]:
stride-0 middle dim is fine, only the innermost step matters for packing).

Schedule notes (measured): exec_time = NTFF span from the first framework
MEMSET to the last postamble instruction; a ~9us NRT/Tile teardown after the
last DMA is constant (full semaphore-range sweep — independent of
instruction count), and the ~5.5us runtime preamble is off the clock.  Small
leading tiles start compute sooner; small trailing tiles shorten the
end-of-pipeline drain.  GPSIMD tensor ops were tried for the tree and are a
net loss (slow per element + SBUF-port contention with DVE).
"""

import numpy as np

N, C = 262144, 256
N_CORES = 8
ROWS_PER_CORE = N // N_CORES        # 32768
P = 128                             # SBUF partitions
ROWS_PER_PART = ROWS_PER_CORE // P  # 256 rows owned by each partition

_NC_CACHE = {}


def _get_nc():
    if "nc" in _NC_CACHE:
        return _NC_CACHE["nc"]
    from contextlib import ExitStack

    import concourse.tile as tile
    from concourse import bacc, mybir

    nc = bacc.Bacc(
        "TRN2", target_bir_lowering=False, debug=False,
        enable_asserts=False, num_devices=N_CORES,
    )
    x_h = nc.dram_tensor("x", [ROWS_PER_CORE, C], mybir.dt.float16,
                         kind="ExternalInput")
    o_h = nc.dram_tensor("out", [ROWS_PER_CORE, C], mybir.dt.float16,
                         kind="ExternalOutput")
    x_prc = x_h.ap().rearrange("(p r) c -> p r c", p=P)
    o_prc = o_h.ap().rearrange("(p r) c -> p r c", p=P)

    # Tile schedule: (rows, act_rows).  Small leading tiles start compute
    # ~4us sooner (pipeline fill); 32-row steady-state tiles halve the DMA
    # count.  act_rows of each tile get their sum from per-row ACT
    # exp+accum; the rest go through the 2x TT add tree on DVE.  32 total
    # ACT-sum rows balances the two engine queues at ~80us each; the last
    # tiles have none so the drain tail stays short.
    SEGS = [(4, 0), (4, 0), (8, 0), (16, 4), (16, 0), (16, 4),
            (32, 6), (32, 6), (32, 0), (32, 6), (32, 6),
            (16, 0), (16, 0)]
    assert sum(r for r, _ in SEGS) == ROWS_PER_PART

    with tile.TileContext(nc) as tc, ExitStack() as ctx:
        xp = ctx.enter_context(tc.tile_pool(name="xp", bufs=4))
        ep = ctx.enter_context(tc.tile_pool(name="ep", bufs=4))
        sp = ctx.enter_context(tc.tile_pool(name="sp", bufs=4))
        cp = ctx.enter_context(tc.tile_pool(name="cp", bufs=1))
        # memset zero-bias tile: a float bias would be materialized as a
        # const tensor whose DMA load serializes ahead of the first input
        # DMA on the HWDGE queue (~6us of kernel-start latency).
        bias0 = cp.tile([P, 1], mybir.dt.float32)
        nc.vector.memset(bias0[:], 0.0)
        # Dummy tiny exp: forces the Exp ACT_TABLE_LOAD to happen now,
        # overlapped with the first input DMA, instead of serialized
        # between the first DMA-wait and the first real exp (~2.3us).
        warm = cp.tile([P, 1], mybir.dt.float16)
        nc.scalar.activation(warm[:], bias0[:],
                             mybir.ActivationFunctionType.Exp, bias=bias0[:])
        W = 2                      # recip replication width for the TT mul
        row0 = 0
        for R_t, k in SEGS:
            xt = xp.tile([P, R_t * C], mybir.dt.float16, tag="xt", bufs=4)
            # inputs issue from sync (HWDGE); outputs from gpsimd (SWDGE)
            # so input DMAs never queue behind output DMAs waiting on the
            # DVE chain (HWDGE is FIFO per issuing engine).
            nc.sync.dma_start(
                xt[:].rearrange("p (r c) -> p r c", c=C),
                x_prc[:, row0:row0 + R_t, :],
            )
            et = ep.tile([P, R_t * C], mybir.dt.float16, tag="et", bufs=4)
            st = sp.tile([P, R_t], mybir.dt.float32, tag="st", bufs=6)
            e3 = et[:].rearrange("p (r c) -> p r c", c=C)
            B = R_t - k            # rows summed via the DVE tree
            # big exp first so the DVE tree can start while the per-row
            # ACT accum rows (the tile's last k rows) are still running
            nc.scalar.activation(
                et[:, 0:B * C], xt[:, 0:B * C],
                mybir.ActivationFunctionType.Exp, bias=bias0[:],
            )
            for r in range(B, R_t):
                nc.scalar.activation(
                    et[:, r * C:(r + 1) * C],
                    xt[:, r * C:(r + 1) * C],
                    mybir.ActivationFunctionType.Exp,
                    bias=bias0[:],
                    accum_out=st[:, r:r + 1],
                )
            eB = e3[:, 0:B, :]
            h1 = sp.tile([P, 32, 128], mybir.dt.float16, tag="h1", bufs=4)
            nc.vector.tensor_add(h1[:, 0:B, :], eB[:, :, 0:128],
                                 eB[:, :, 128:256])
            h2 = sp.tile([P, 32, 64], mybir.dt.float16, tag="h2", bufs=4)
            nc.vector.tensor_add(h2[:, 0:B, :], h1[:, 0:B, 0:64],
                                 h1[:, 0:B, 64:128])
            h3 = sp.tile([P, 32, 32], mybir.dt.float16, tag="h3", bufs=4)
            nc.vector.tensor_add(h3[:, 0:B, :], h2[:, 0:B, 0:32],
                                 h2[:, 0:B, 32:64])
            h4 = sp.tile([P, 32, 16], mybir.dt.float16, tag="h4", bufs=4)
            nc.vector.tensor_add(h4[:, 0:B, :], h3[:, 0:B, 0:16],
                                 h3[:, 0:B, 16:32])
            nc.vector.reduce_sum(st[:, 0:B], h4[:, 0:B, :],
                                 axis=mybir.AxisListType.X)
            # 1/sum as fp16, replicated to W consecutive elements so the
            # normalization runs as ONE full-tile TENSOR_TENSOR in the 2x
            # packed mode (in1 = [P,(R,W),(C/W,0),(W,1)]: stride-0 middle
            # dim is fine, only the innermost step matters for packing).
            rb = sp.tile([P, 32, W], mybir.dt.float16, tag="rb", bufs=6)
            with nc.allow_low_precision(reason="fp16 softmax kernel"):
                nc.vector.reciprocal(rb[:, 0:R_t, 0:1], st[:][:, :, None])
            nc.vector.tensor_copy(
                rb[:, 0:R_t, 1:W],
                rb[:, 0:R_t, 0:1].broadcast_to((P, R_t, W - 1)),
            )
            e4 = et[:].rearrange("p (r k w) -> p r k w", w=W, k=C // W)
            in1 = rb[:, 0:R_t, None, :].broadcast_to((P, R_t, C // W, W))
            nc.vector.tensor_mul(e4, e4, in1)
            nc.gpsimd.dma_start(o_prc[:, row0:row0 + R_t, :], e3)
            row0 += R_t
    nc.compile()
    _NC_CACHE["nc"] = nc
    return nc


def _install_ntff_hook():
    """Make the optional antenv.axon_hooks module available so the
    trace=True / BASS_TRACE path of run_bass_kernel_spmd works under axon
    (the image's antenv package lacks axon_hooks; boot() skips the NTFF
    hook registration silently in that case)."""
    import sys
    import types

    try:
        import antenv.axon_hooks  # noqa: F401
    except ImportError:
        try:
            import antenv
        except ImportError:
            return
        mod = types.ModuleType("antenv.axon_hooks")
        holder = {}
        mod.set_axon_ntff_profile_hook = lambda h: holder.__setitem__("h", h)
        mod.get_axon_ntff_profile_hook = lambda: holder.get("h")
        sys.modules["antenv.axon_hooks"] = mod
        antenv.axon_hooks = mod
    from antenv.axon_hooks import (
        get_axon_ntff_profile_hook,
        set_axon_ntff_profile_hook,
    )

    if get_axon_ntff_profile_hook() is None:
        try:
            from trn_agent_boot.trn_boot import _ntff_profile_via_ctypes

            set_axon_ntff_profile_hook(
                _ntff_profile_via_ctypes("/opt/axon/libaxon_pjrt.so")
            )
        except Exception:
            pass


def _build_per_device_runner(nc):
    """Per-device dispatch in HBM-domain-interleaved order.

    Host->device uploads serialize, so with a single global dispatch each
    even core's NEFF executes exactly while its HBM-domain partner's
    (core+1) input upload streams into the same HBM stack.  Dispatching
    per-device executions in order 0,2,4,6,1,3,5,7 makes the upload that
    overlaps core i's execution always target a different HBM domain.
    """
    import jax
    import jax.numpy as jnp
    from concourse import bass2jax, mybir

    bass2jax.install_neuronx_cc_hook()

    partition_name = (
        nc.partition_id_tensor.name if nc.partition_id_tensor else None
    )
    in_names, out_names, out_avals = [], [], []
    for alloc in nc.m.functions[0].allocations:
        if not isinstance(alloc, mybir.MemoryLocationSet):
            continue
        assert alloc.memorylocations
        name = alloc.memorylocations[0].name
        if alloc.kind == "ExternalInput":
            if name != partition_name:
                in_names.append(name)
        elif alloc.kind == "ExternalOutput":
            assert alloc.tensor_shape is not None and alloc.dtype is not None
            out_names.append(name)
            out_avals.append(
                jax.core.ShapedArray(
                    tuple(alloc.tensor_shape), mybir.dt.np(alloc.dtype)
                )
            )
    n_params = len(in_names)
    all_in_names = tuple(in_names) + tuple(out_names)
    if partition_name is not None:
        # supplied as the last operand via PartitionIdOp, mirroring
        # run_bass_via_pjrt; this program never reads it (no collectives).
        all_in_names = all_in_names + (partition_name,)

    def _body(*args):
        operands = list(args)
        if partition_name is not None:
            operands.append(bass2jax.partition_id_tensor())
        outs = bass2jax._bass_exec_p.bind(
            *operands,
            out_avals=tuple(out_avals),
            in_names=all_in_names,
            out_names=tuple(out_names),
            lowering_input_output_aliases=(),
            sim_require_finite=True,
            sim_require_nnan=True,
            nc=nc,
        )
        return tuple(outs)

    donate = tuple(range(n_params, n_params + len(out_names)))
    jitted = jax.jit(_body, donate_argnums=donate, keep_unused=True)

    devs = jax.devices()[:N_CORES]
    zeros_makers = {
        d: jax.jit(
            lambda: tuple(jnp.zeros(a.shape, a.dtype) for a in out_avals),
            out_shardings=jax.sharding.SingleDeviceSharding(devs[d]),
        )
        for d in range(N_CORES)
    }

    def run(in_maps, order=(0, 2, 4, 6, 1, 3, 5, 7)):
        futures = {}
        for d in order:
            args = [
                jax.device_put(np.asarray(in_maps[d][n]), devs[d])
                for n in in_names
            ]
            zeros = zeros_makers[d]()  # created on-device: no H2D traffic
            futures[d] = jitted(*args, *zeros)
        return [
            {n: np.asarray(futures[d][i]) for i, n in enumerate(out_names)}
            for d in range(len(in_maps))
        ]

    return run


def _run(x, **spmd_kwargs):
    _install_ntff_hook()
    nc = _get_nc()
    x = np.asarray(x)
    assert x.shape == (N, C), x.shape
    x16 = np.ascontiguousarray(x, dtype=np.float16)
    shards = np.split(x16, N_CORES, axis=0)
    in_maps = [{"x": np.ascontiguousarray(s)} for s in shards]

    if not spmd_kwargs:
        try:
            if "runner" not in _NC_CACHE:
                _NC_CACHE["runner"] = _build_per_device_runner(nc)
            results = _NC_CACHE["runner"](in_maps)
            out = np.concatenate(
                [r["out"] for r in results], axis=0
            ).astype(np.float32)
            return out, None
        except Exception:
            pass  # fall back to the stock global-dispatch path

    from concourse.bass_utils import run_bass_kernel_spmd

    res = run_bass_kernel_spmd(
        nc, in_maps, core_ids=list(range(N_CORES)), **spmd_kwargs
    )
    out = np.concatenate(
        [np.asarray(res.results[i]["out"]) for i in range(N_CORES)], axis=0
    ).astype(np.float32)
    return out, res


def kernel(x):
    return _run(x)[0]
